# revision 29
# baseline (speedup 1.0000x reference)
"""Trainium2 Bass kernel: 8-head attention block (BN-folded projections,
relative-position bias, softmax, GELU + output projection).

Sharding: data-parallel over batch across 8 NeuronCores (2 batch elems/core).
All weights / bias tables replicated; no collectives.

v3 design (engine-balanced around the ScalarE exp floor ~144us, with
software pipelining to kill the startup/steady/tail stalls of v2):
  XT  [c=256, n=1024]  bf16   (host pre-transposed, pre-cast)
  QT/KT [64, n] tiles, 2 heads each at offsets 0/32 (matmul operands must
        start at partition 0/32/64)
  V    [n, (h, 65)]    bf16   65th column = 1.0 -> AV matmul row 64 yields
                              the softmax denominator for free
  dots: per head, K=32 plain matmul -> psum [j, i]
  relative-position bias applied MULTIPLICATIVELY after exp:
      exp(scale*dots + b/scale) = exp(scale*dots) * exp(b/scale)
  with exp(b/scale) a host-precomputed bf16 table; the elementwise multiply
  is split DVE/GpSimd (2 of 5 on GpSimd) to stay under the ScalarE exp time.
  normalize: reciprocal of psum row 64 (bf16), replicated across the head
  pair's 128 partitions by two accumulating K=1 masked matmuls; one DVE
  multiply per half writes the gelu input tile.

  Pipelining: engines execute their queues in order, so emission order is
  schedule. Only the (b0, tt0) Q/K projection precedes the first combo; all
  other projections and the V projection are "filler" units pumped into the
  dots stream. Each combo's AV+normalize is deferred into the NEXT combo's
  jt loop (so the next dots feed ScalarE while AV runs). ebias DMAs issue
  on the GpSimd and SP queues (split), xt on the DVE queue, to parallelize
  the DMA drip. b0's GELU+out-projection overlaps the last combo's AV.

  exp and gelu are forced into disjoint program phases so the ScalarE
  activation table loads exactly twice.
"""

import os
import numpy as np
import ml_dtypes

import concourse.bass as bass
import concourse.tile as tile
from concourse import bacc, mybir
from concourse.bass_utils import run_bass_kernel_spmd
from concourse.tile import add_dep_helper

NPBF16 = ml_dtypes.bfloat16
BF16 = mybir.dt.bfloat16
F32 = mybir.dt.float32

HEADS, DK, DV = 8, 32, 64
N = 1024          # positions = 32*32
C = 256           # channels
IDV = HEADS * DV  # 512
NCORES = 8
BLOC = 2          # batch elems per core
SCALE = float(DK) ** -0.5
EPS = 1e-5

_CACHE = {}


def _build_nc():
    nc = bacc.Bacc("TRN2", target_bir_lowering=False, debug=False)

    xt_d = nc.declare_dram_parameter("xt", [BLOC, 2, 128, N], BF16, isOutput=False)
    wq_d = nc.declare_dram_parameter("wq", [128, 2, C], BF16, isOutput=False)
    wk_d = nc.declare_dram_parameter("wk", [128, 2, C], BF16, isOutput=False)
    wv_d = nc.declare_dram_parameter("wv", [128, 2, IDV], BF16, isOutput=False)
    wo_d = nc.declare_dram_parameter("wo", [128, 4, C], BF16, isOutput=False)
    oq_d = nc.declare_dram_parameter("oq", [128, 2], F32, isOutput=False)
    ok_d = nc.declare_dram_parameter("ok", [128, 2], F32, isOutput=False)
    ovg_d = nc.declare_dram_parameter("ovg", [128, 4], F32, isOutput=False)
    bout_d = nc.declare_dram_parameter("bout", [128, C], F32, isOutput=False)
    # ebias[hg, is, jt, j1, h, i1] = exp(pos_bias[j, i, 4*hg+h] / SCALE)
    ebias_d = nc.declare_dram_parameter("ebias", [2, 2, 8, 128, 4, 512], BF16,
                                        isOutput=False)
    out_d = nc.declare_dram_parameter("out", [BLOC, N, C], F32, isOutput=True)

    Exp = mybir.ActivationFunctionType.Exp
    Gelu = mybir.ActivationFunctionType.Gelu

    with tile.TileContext(nc) as tc:
        with (
            tc.tile_pool(name="const", bufs=1) as const,
            tc.tile_pool(name="persist", bufs=1) as persist,
            tc.tile_pool(name="ebp", bufs=12) as ebp,
            tc.tile_pool(name="expp", bufs=30) as expp,
            tc.tile_pool(name="recp", bufs=6) as recp,
            tc.tile_pool(name="rsbp", bufs=3) as rsbp,
            tc.tile_pool(name="outp", bufs=6) as outp,
            tc.tile_pool(name="dpsum", bufs=2, space="PSUM") as dpsum,
            tc.tile_pool(name="avpsum", bufs=3, space="PSUM") as avpsum,
            tc.tile_pool(name="reppsum", bufs=1, space="PSUM") as reppsum,
        ):
            # ---------------- constants (SP queue, proj weights first) -----
            wq_s = const.tile([128, 2, C], BF16, tag="wq")
            nc.sync.dma_start(wq_s[:], wq_d[:])
            wk_s = const.tile([128, 2, C], BF16, tag="wk")
            nc.sync.dma_start(wk_s[:], wk_d[:])
            # xt split across the GpSimd and SP DMA queues so both ct halves
            # of b0 land ~3.5us in and the first projection can start
            xt = {}
            for b in range(BLOC):
                for ct in range(2):
                    t = persist.tile([128, N], BF16, tag=f"xt{b}{ct}",
                                     name=f"xt{b}{ct}")
                    xt[b, ct] = t
            nc.gpsimd.dma_start(xt[0, 0][:, 0:512], xt_d[0, 0][:, 0:512])
            nc.sync.dma_start(xt[0, 1][:, 0:512], xt_d[0, 1][:, 0:512])
            nc.gpsimd.dma_start(xt[0, 0][:, 512:N], xt_d[0, 0][:, 512:N])
            nc.sync.dma_start(xt[0, 1][:, 512:N], xt_d[0, 1][:, 512:N])
            oq_s = const.tile([128, 2], F32, tag="oq")
            nc.sync.dma_start(oq_s[:], oq_d[:])
            ok_s = const.tile([128, 2], F32, tag="ok")
            nc.sync.dma_start(ok_s[:], ok_d[:])
            wv_s = const.tile([128, 2, IDV], BF16, tag="wv")
            nc.sync.dma_start(wv_s[:], wv_d[:])
            nc.gpsimd.dma_start(xt[1, 0][:], xt_d[1, 0])
            nc.sync.dma_start(xt[1, 1][:], xt_d[1, 1])
            ovg_s = const.tile([128, 4], F32, tag="ovg")
            nc.sync.dma_start(ovg_s[:], ovg_d[:])
            wo_s = const.tile([128, 4, C], BF16, tag="wo")
            nc.sync.dma_start(wo_s[:], wo_d[:])
            bout_s = const.tile([128, C], F32, tag="bout")
            nc.sync.dma_start(bout_s[:], bout_d[:])
            # replication masks: rmask[0] selects partitions 0..63, [1] 64..127
            rmask = const.tile([1, 2, 128], BF16, tag="rmask")
            nc.vector.memset(rmask[:], 0.0)
            nc.vector.memset(rmask[:, 0, 0:64], 1.0)
            nc.vector.memset(rmask[:, 1, 64:128], 1.0)

            # persistent tiles
            qt, kt, vt, gelu_t = {}, {}, {}, {}
            for b in range(BLOC):
                for g in range(4):
                    qt[b, g] = persist.tile([64, N], BF16, tag=f"qt{b}{g}",
                                            name=f"qt{b}{g}")
                    kt[b, g] = persist.tile([64, N], BF16, tag=f"kt{b}{g}",
                                            name=f"kt{b}{g}")
                for it in range(8):
                    vt[b, it] = persist.tile([128, 8, 65], BF16, tag=f"v{b}{it}",
                                             name=f"v{b}{it}")
                for dt in range(4):
                    gelu_t[b, dt] = persist.tile([128, N], BF16, tag=f"g{b}{dt}",
                                                 name=f"g{b}{dt}")

            # ---------------- emission helpers ----------------
            # Engine queues execute in emission order, and reads of tiles
            # whose writers are not yet EMITTED see garbage. `written` guards
            # that invariant.
            written = set()

            def emit_proj_qk(b, tt, i2, use_dps=False):
                """One Q+K projection unit: psums + stripe adds."""
                for wsb, osb, dst, tag in ((wq_s, oq_s, qt, "q"),
                                           (wk_s, ok_s, kt, "k")):
                    if use_dps:
                        ps = dpsum.tile([128, 2, 512], F32, tag="dps")
                        ps = ps[:, 0, :]
                    else:
                        ps = reppsum.tile([128, 512], F32, tag="rep")
                    for ct in range(2):
                        nc.tensor.matmul(
                            ps[:],
                            wsb[:, ct, tt * 128:(tt + 1) * 128],
                            xt[b, ct][:, i2 * 512:(i2 + 1) * 512],
                            start=(ct == 0), stop=(ct == 1),
                        )
                    for gh in range(2):
                        nc.vector.tensor_scalar_add(
                            dst[b, 2 * tt + gh][:, i2 * 512:(i2 + 1) * 512],
                            ps[64 * gh:64 * gh + 64, :],
                            osb[64 * gh:64 * gh + 64, tt:tt + 1])
                        written.add((tag, b, 2 * tt + gh, i2))

            def emit_proj_v(b, it):
                """One V projection unit -> vt[b, it] with ones column."""
                v = vt[b, it]
                ps = reppsum.tile([128, 512], F32, tag="rep")
                for ct in range(2):
                    nc.tensor.matmul(
                        ps[:],
                        xt[b, ct][:, it * 128:(it + 1) * 128],
                        wv_s[:, ct, :],
                        start=(ct == 0), stop=(ct == 1),
                    )
                nc.vector.memset(v[:, :, 64:65], 1.0)
                nc.vector.tensor_copy(
                    v[:, :, 0:64],
                    ps[:].rearrange("p (h d) -> p h d", h=8))
                written.add(("v", b, it))

            fillers = []

            def pump(n):
                for _ in range(n):
                    if fillers:
                        fillers.pop(0)()

            last_exp = [None]
            eb_groups = {}

            def emit_eb_group(hg, isl):
                ebs = {}
                for jt in range(8):
                    bt = ebp.tile([128, 4, 512], BF16, tag="eb",
                                  name=f"eb{hg}{isl}{jt}")
                    # split the 8 tile loads across two DMA queues
                    eng = nc.gpsimd if jt % 2 == 0 else nc.sync
                    eng.dma_start(bt[:], ebias_d[hg, isl, jt])
                    ebs[jt] = bt
                eb_groups[hg, isl] = ebs

            av_state = {}

            def emit_av_half(combo, pair, half):
                """AV matmul chain for one head of a finished combo."""
                hg, isl, b, et_t = combo
                h = 2 * pair + half
                hglob = 4 * hg + h
                for jt in range(8):
                    assert ("v", b, jt) in written
                av = avpsum.tile([128, 512], F32, tag="av")
                for jt in range(8):
                    nc.tensor.matmul(
                        av[0:65, :],
                        vt[b, jt][:, hglob, :],
                        et_t[jt, pair][:, half, :],
                        start=(jt == 0), stop=(jt == 7),
                    )
                rec = recp.tile([1, 512], BF16, tag="rec")
                with nc.allow_low_precision(reason="softmax reciprocal bf16"):
                    nc.vector.reciprocal(rec[:], av[64:65, :])
                av_state[pair, half] = (av, rec)

            def emit_av_norm(combo, pair):
                """Replicate reciprocals, write gelu input for a head pair."""
                hg, isl, b, et_t = combo
                i0 = isl * 512
                rep_ps = reppsum.tile([128, 512], F32, tag="rep")
                for half in range(2):
                    nc.tensor.matmul(
                        rep_ps[:], rmask[:, half, :],
                        av_state[pair, half][1][:],
                        start=(half == 0), stop=(half == 1),
                    )
                rep_sb = rsbp.tile([128, 512], F32, tag="rsb")
                nc.vector.tensor_copy(rep_sb[:], rep_ps[:])
                dt = 2 * hg + pair
                for half in range(2):
                    nc.vector.tensor_mul(
                        gelu_t[b, dt][64 * half:64 * half + 64, i0:i0 + 512],
                        av_state[pair, half][0][0:64, :],
                        rep_sb[64 * half:64 * half + 64, :])

            def emit_av_block(combo, pair):
                for half in range(2):
                    emit_av_half(combo, pair, half)
                emit_av_norm(combo, pair)

            # ---------------- the pipelined combo stream ----------------
            # prologue: Q/K (b0,tt0) first i2-half (dots c1-jt0..3 need just
            # that) plus half the b0 V projection while everything is idle
            # prologue with gh-interleaved stripes: the first dots pair only
            # needs the gh=0 rows of qt/kt, so those two adds come first
            pro_ps = {}
            for wsb, osb, tag in ((wq_s, oq_s, "q"), (wk_s, ok_s, "k")):
                ps = dpsum.tile([128, 2, 512], F32, tag="dps",
                                name=f"pro{tag}")
                pro_ps[tag] = (ps, osb)
                for ct in range(2):
                    nc.tensor.matmul(
                        ps[:, 0, :], wsb[:, ct, 0:128],
                        xt[0, ct][:, 0:512],
                        start=(ct == 0), stop=(ct == 1),
                    )
            for gh in range(2):
                for wsb, osb, dst, tag in ((wq_s, oq_s, qt, "q"),
                                           (wk_s, ok_s, kt, "k")):
                    ps, osb2 = pro_ps[tag]
                    nc.vector.tensor_scalar_add(
                        dst[0, gh][:, 0:512],
                        ps[64 * gh:64 * gh + 64, 0, :],
                        osb2[64 * gh:64 * gh + 64, 0:1])
                    written.add((tag, 0, gh, 0))

            fillers.extend([
                lambda: emit_proj_qk(0, 0, 1),   # dots c1-jt4
                lambda: emit_proj_v(0, 0),
                lambda: emit_proj_v(0, 1),
                lambda: emit_proj_v(0, 2),
                lambda: emit_proj_v(0, 3),
                lambda: emit_proj_v(0, 4),
                lambda: emit_proj_v(0, 5),
                lambda: emit_proj_v(0, 6),
                lambda: emit_proj_v(0, 7),       # all V(b0) before AV(c1)
                lambda: emit_proj_qk(0, 1, 0),   # dots c3-jt0
                lambda: emit_proj_v(1, 0),
                lambda: emit_proj_v(1, 1),
                lambda: emit_proj_qk(0, 1, 1),   # dots c3-jt4
                lambda: emit_proj_v(1, 2),
                lambda: emit_proj_v(1, 3),
                lambda: emit_proj_qk(1, 1, 0),   # dots c5-jt0
                lambda: emit_proj_qk(1, 1, 1),   # dots c5-jt4
                lambda: emit_proj_v(1, 4),
                lambda: emit_proj_qk(1, 0, 0),   # dots c7-jt0
                lambda: emit_proj_v(1, 5),
                lambda: emit_proj_v(1, 6),
                lambda: emit_proj_v(1, 7),       # all V(b1) before AV(c5)
                lambda: emit_proj_qk(1, 0, 1),   # dots c7-jt4
            ])

            # b-outer, hg/isl mirrored so c4 and c5 share an ebias group
            combos = [(0, 0, 0), (0, 0, 1), (0, 1, 0), (0, 1, 1),
                      (1, 1, 1), (1, 1, 0), (1, 0, 1), (1, 0, 0)]
            combos = [(hg, isl, b) for b, hg, isl in combos]
            eb_by_ci = {}

            def load_eb(ci):
                hg, isl, _ = combos[ci]
                if ci > 0 and combos[ci - 1][:2] == (hg, isl):
                    eb_by_ci[ci] = eb_by_ci[ci - 1]
                    return
                ebs = {}
                for jt in range(8):
                    bt = ebp.tile([128, 4, 512], BF16, tag="eb",
                                  name=f"eb{ci}{jt}")
                    nc.sync.dma_start(bt[:], ebias_d[hg, isl, jt])
                    ebs[jt] = bt
                eb_by_ci[ci] = ebs

            load_eb(0)
            prev = None
            for ci, (hg, isl, b) in enumerate(combos):
                last = ci == len(combos) - 1
                if not last:
                    load_eb(ci + 1)
                ebs = eb_by_ci[ci]
                i0 = isl * 512
                et_t = {}
                unit = [0]

                def emit_unit(pair, jt):
                    dps = dpsum.tile([128, 2, 512], F32, tag="dps")
                    for half in range(2):
                        h = 2 * pair + half
                        hglob = 4 * hg + h
                        g, off = hglob // 2, 32 * (hglob % 2)
                        assert ("q", b, g, i0 // 512) in written
                        assert ("k", b, g, jt // 4) in written
                        nc.tensor.matmul(
                            dps[:, half, :],
                            kt[b, g][off:off + 32, jt * 128:(jt + 1) * 128],
                            qt[b, g][off:off + 32, i0:i0 + 512],
                            start=True, stop=True,
                        )
                    et = expp.tile([128, 2, 512], BF16, tag="exp",
                                   name=f"exp{ci}{jt}{pair}")
                    et_t[jt, pair] = et
                    ae = nc.scalar.activation(et[:], dps[:], Exp, scale=SCALE)
                    last_exp[0] = ae
                    # GpSimd takes pair-1 multiplies only: the next combo's
                    # pair-0 AV (at jt4) never waits on the slow engine. The
                    # last combo's final pair drains on DVE for a short tail.
                    pool_set = (1, 3, 5) if last else (1, 3, 5, 9, 11, 13)
                    eng = (nc.gpsimd if unit[0] % 16 in pool_set
                           else nc.vector)
                    unit[0] += 1
                    eng.tensor_mul(et[:], et[:],
                                   ebs[jt][:, 2 * pair:2 * pair + 2, :])

                pump_jts = {0: {0: 2, 2: 2, 4: 2, 6: 2, 7: 1},
                            1: {0: 1, 2: 1, 4: 1},
                            2: {0: 1, 2: 1, 4: 1},
                            3: {0: 1, 2: 1, 4: 1},
                            4: {0: 1, 2: 1, 4: 1},
                            5: {0: 1, 2: 1}}.get(ci, {})
                if not last:
                    # AV of the previous combo rides in single-head chains at
                    # the odd jts: each 1.7us chunk fits inside the ~2.1us of
                    # slack the 2-slot dots-psum ring can buffer, so ScalarE
                    # never starves. Fillers pump at even jts.
                    for jt in range(8):
                        for pair in range(2):
                            emit_unit(pair, jt)
                        pump(pump_jts.get(jt, 0))
                        if prev is not None:
                            if jt == 1:
                                emit_av_half(prev, 0, 0)
                            elif jt == 3:
                                emit_av_half(prev, 0, 1)
                                emit_av_norm(prev, 0)
                            elif jt == 5:
                                emit_av_half(prev, 1, 0)
                            elif jt == 6:
                                emit_av_half(prev, 1, 1)
                                emit_av_norm(prev, 1)
                else:
                    # pair-major: the combo's own pair-0 AV overlaps its
                    # pair-1 exps, shortening the drain tail
                    cur = (hg, isl, b, et_t)
                    fin_av = {}
                    fin_rec = {}

                    def emit_fin_acc(jt):
                        # AV accumulation for the final pair, lagged 2 jts so
                        # the multiplies are surely drained
                        for half in range(2):
                            hglob = 4 * hg + 2 + half
                            if jt == 0:
                                favt = avpsum.tile([128, 512], F32, tag="av",
                                                   name=f"finav{half}")
                                fin_av[half] = favt
                            nc.tensor.matmul(
                                fin_av[half][0:65, :],
                                vt[b, jt][:, hglob, :],
                                et_t[jt, 1][:, half, :],
                                start=(jt == 0), stop=(jt == 7),
                            )
                            if jt == 7:
                                rec = recp.tile([1, 512], BF16, tag="rec")
                                fin_rec[half] = rec
                                with nc.allow_low_precision(
                                        reason="softmax reciprocal bf16"):
                                    nc.vector.reciprocal(
                                        rec[:], fin_av[half][64:65, :])
                                av_state[1, half] = (fin_av[half], rec)

                    for pair in range(2):
                        for jt in range(8):
                            emit_unit(pair, jt)
                            if pair == 1 and jt >= 3:
                                emit_fin_acc(jt - 3)
                            if pair == 0 and prev is not None:
                                if jt == 1:
                                    emit_av_half(prev, 0, 0)
                                elif jt == 3:
                                    emit_av_half(prev, 0, 1)
                                    emit_av_norm(prev, 0)
                                elif jt == 5:
                                    emit_av_half(prev, 1, 0)
                                elif jt == 6:
                                    emit_av_half(prev, 1, 1)
                                    emit_av_norm(prev, 1)
                            elif pair == 1:
                                if jt == 1:
                                    emit_av_half(cur, 0, 0)
                                elif jt == 2:
                                    emit_av_half(cur, 0, 1)
                                    emit_av_norm(cur, 0)
                prev = (hg, isl, b, et_t)
            for jt in (5, 6, 7):
                emit_fin_acc(jt)
            emit_av_norm(prev, 1)
            assert not fillers, f"{len(fillers)} fillers unpumped"

            # ---------------- GELU + output projection ----------------
            # gelu runs in 512-column halves ordered by data readiness, and
            # out-projection it-blocks start as soon as the halves they read
            # are done; psum alternates the dps/av rings and the output DMAs
            # ride the idle GpSimd/SP queues.
            def emit_gelu_half(b, dt, ih):
                gi = nc.scalar.activation(
                    gelu_t[b, dt][:, ih * 512:(ih + 1) * 512],
                    gelu_t[b, dt][:, ih * 512:(ih + 1) * 512], Gelu,
                    bias=ovg_s[:, dt:dt + 1], scale=1.0)
                if last_exp[0] is not None:
                    add_dep_helper(gi.ins, last_exp[0].ins, sync=False,
                                   reason="group ACT table sets")

            def emit_outproj_block(b, it, slot):
                if slot % 2 == 0:
                    ops = dpsum.tile([128, 2, 512], F32, tag="dps")
                    ops = ops[:, 0, 0:C]
                else:
                    ops = avpsum.tile([128, 512], F32, tag="av")
                    ops = ops[:, 0:C]
                for dt in range(4):
                    nc.tensor.matmul(
                        ops,
                        gelu_t[b, dt][:, it * 128:(it + 1) * 128],
                        wo_s[:, dt, :],
                        start=(dt == 0), stop=(dt == 3),
                    )
                osb = outp.tile([128, C], F32, tag="osb")
                nc.vector.tensor_add(osb[:], ops, bout_s[:])
                eng = nc.gpsimd if slot % 2 == 0 else nc.sync
                eng.dma_start(out_d[b, it * 128:(it + 1) * 128, :], osb[:])

            def emit_gelu_full(b, dt):
                gi = nc.scalar.activation(gelu_t[b, dt][:], gelu_t[b, dt][:],
                                          Gelu, bias=ovg_s[:, dt:dt + 1],
                                          scale=1.0)
                if last_exp[0] is not None:
                    add_dep_helper(gi.ins, last_exp[0].ins, sync=False,
                                   reason="group ACT table sets")

            for dt in range(4):
                emit_gelu_full(0, dt)
            emit_gelu_full(1, 2)
            emit_gelu_full(1, 3)
            # dt0/dt1 in halves, the half needing the very last norm dead last
            for dt, ih in ((0, 1), (1, 1), (0, 0), (1, 0)):
                emit_gelu_half(1, dt, ih)
            slot = 0
            for it in range(8):
                emit_outproj_block(0, it, slot)
                slot += 1
            for it in (4, 5, 6, 7, 0, 1, 2, 3):
                emit_outproj_block(1, it, slot)
                slot += 1

    nc.compile()
    return nc


def _host_prep(x, w_q, bn_q, w_k, bn_k, w_v, bn_v, w_out, b_out, bn_out,
               pos_table):
    """Fold BN into weights, build exp-bias table, shard across cores."""
    def fold(bn):
        g, b_, m, v = [np.asarray(a, np.float64) for a in bn]
        s = g / np.sqrt(v + EPS)
        return s, b_ - m * s

    sq, oq = fold(bn_q)
    sk, ok = fold(bn_k)
    sv, ov = fold(bn_v)
    so, oo = fold(bn_out)

    def wtile(w, s, ncols):
        # [C_in, D] * s[D] -> [128, C_in//128, D] bf16 (partition-major)
        w_eff = (np.asarray(w, np.float64) * s[None, :]).astype(np.float32)
        return np.ascontiguousarray(
            w_eff.reshape(-1, 128, ncols).transpose(1, 0, 2)).astype(NPBF16)

    wq = wtile(w_q, sq, C)
    wk = wtile(w_k, sk, C)
    wv = wtile(w_v, sv, IDV)
    wo = wtile(w_out, so, C)

    oq_t = np.ascontiguousarray(oq.astype(np.float32).reshape(2, 128).T)
    ok_t = np.ascontiguousarray(ok.astype(np.float32).reshape(2, 128).T)
    ovg_t = np.ascontiguousarray(ov.astype(np.float32).reshape(4, 128).T)
    bout_eff = (np.asarray(b_out, np.float64) * so + oo).astype(np.float32)
    bout_t = np.ascontiguousarray(np.broadcast_to(bout_eff, (128, C)))

    # multiplicative position-bias table: exp(bias[j, i, h] / SCALE)
    r = np.arange(32)
    pos = np.stack(np.meshgrid(r, r, indexing="ij"), axis=-1).reshape(-1, 2)
    rel = np.abs(pos[:, None, :] - pos[None, :, :])
    idx = rel[..., 0] * 32 + rel[..., 1]           # [n, n]
    bias = np.asarray(pos_table, np.float64)[idx]  # [j, i, 8]
    ebias = np.exp(bias / SCALE).astype(np.float32)
    # -> [hg, is, jt, j1, h, i1]
    ebias = ebias.reshape(8, 128, 2, 512, 2, 4)    # jt, j1, is, i1, hg, h
    ebias = np.ascontiguousarray(
        ebias.transpose(4, 2, 0, 1, 5, 3)).astype(NPBF16)

    x = np.asarray(x, np.float32).reshape(-1, N, C)      # [B, n, C]
    common = dict(wq=wq, wk=wk, wv=wv, wo=wo, oq=oq_t, ok=ok_t, ovg=ovg_t,
                  bout=bout_t, ebias=ebias)
    in_maps = []
    for c in range(NCORES):
        xl = x[c * BLOC:(c + 1) * BLOC]                  # [2, n, C]
        xtl = xl.transpose(0, 2, 1).reshape(BLOC, 2, 128, N).astype(NPBF16)
        in_maps.append(dict(common, xt=np.ascontiguousarray(xtl)))
    return in_maps


def kernel(**inputs):
    if "nc" not in _CACHE:
        _CACHE["nc"] = _build_nc()
    nc = _CACHE["nc"]
    in_maps = _host_prep(**inputs)
    res = run_bass_kernel_spmd(nc, in_maps, core_ids=list(range(NCORES)),
                               trace=bool(int(os.environ.get("KTRACE", "0"))))
    _CACHE["last_result"] = res
    outs = [res.results[c]["out"].reshape(BLOC, 32, 32, C)
            for c in range(NCORES)]
    return np.concatenate(outs, axis=0).astype(np.float32)


if __name__ == "__main__":
    nc = _build_nc()
    print("build + compile OK")
